# revision 21
# baseline (speedup 1.0000x reference)
"""CTDG encoder (exp-decay memory GNN) on 8 Trainium2 NeuronCores.

Strategy (pure node-parallel, per the natural sharding of this module):
- Host: shard the 200k nodes into 8 contiguous ranges of 25000 (padded to
  25600 = 25*1024), route each event (unique_sources row) to its owning
  shard, and permute each shard so event nodes come first.  The event
  region is padded to a uniform multiple of 1024 with identity events
  (msg=0, ts=last_update), so every 1024-node pair of device tiles is
  either fully "event" or fully "plain".  memory/static_emb/messages are
  pre-transposed to feature-major [128, nodes] (bf16) so the device never
  transposes.
- Device (SPMD, identical program, per-core data):
  Pass A: per-node scalars in pair-row layout [25, 1024] (f32 math):
      decay = exp((lu - ts)/30), rc = 1/(cnt_new + eps),
      ds = (1 - e_lamb) * exp((upd_lu - now)/30)   (as exp(x/30 + bias))
    then round-tripped through DRAM (bf16) so pass B can fetch them as
    partition-0 rows.
  Pass B: for each of 25 pairs (1024 nodes):
      rc/ds broadcast to [128,1024] SBUF via GPSIMD partition_broadcast
      (uint32-bitcast to halve element count), decay broadcast via two
      K=1 bf16 matmuls (PE), event update + count-normalize + output
      combine on DVE (bf16 2x, 1024-wide), two-layer MLP on PE (bf16,
      512-wide into paired PSUM banks), LeakyReLU (+bias) on ACT
      (1024-wide), IO in 5-pair chunked DMAs.
- Host: inverse-permute, upcast, and concatenate shard outputs.
"""

import numpy as np
import ml_dtypes

import concourse.bacc as bacc
import concourse.tile as tile
from concourse import mybir
from concourse.bass_utils import run_bass_kernel_spmd

N_NODES = 200000
D = 128
NCORES = 8
S = N_NODES // NCORES          # 25000 real nodes per core
TILE = 512                     # matmul / PSUM-bank granularity
PAIR = 1024                    # elementwise granularity
NP = 25                        # pairs per core
S_PAD = NP * PAIR              # 25600
CHP = 5                        # pairs per IO chunk
NCH = NP // CHP                # 5 chunks
CHW = CHP * PAIR               # 5120 columns per chunk
LAMB = 30.0                    # memory-updater decay constant
OUTPUT = 30.0                  # embedding time-decay constant
EPS = 1e-10
SLOPE = 0.01

F32 = mybir.dt.float32
BF16 = mybir.dt.bfloat16
U32 = mybir.dt.uint32
NP_BF16 = ml_dtypes.bfloat16


def _build(NEP, e_lamb, now_time):
    """Build the per-core bass program. NEP = number of event pairs."""
    nc = bacc.Bacc("TRN2", target_bir_lowering=False, debug=False,
                   num_devices=NCORES)
    E_PAD = NEP * PAIR

    msumT_d = nc.dram_tensor("msumT", [D, S_PAD], BF16, kind="ExternalInput")
    # staticT is pre-scaled by e_lamb on the host (constant folding)
    staticT_d = nc.dram_tensor("staticT", [D, S_PAD], BF16, kind="ExternalInput")
    msgT_d = nc.dram_tensor("msgT", [D, E_PAD], BF16, kind="ExternalInput")
    lu_d = nc.dram_tensor("lu_t", [NP, PAIR], F32, kind="ExternalInput")
    ts_d = nc.dram_tensor("ts_t", [NEP, PAIR], F32, kind="ExternalInput")
    cnt_d = nc.dram_tensor("cnt_t", [NP, PAIR], F32, kind="ExternalInput")
    msgc_d = nc.dram_tensor("msgc_t", [NEP, PAIR], F32, kind="ExternalInput")
    w1a_d = nc.dram_tensor("w1a", [D, D], BF16, kind="ExternalInput")
    w1b_d = nc.dram_tensor("w1b", [D, D], BF16, kind="ExternalInput")
    w2_d = nc.dram_tensor("w2", [D, D], BF16, kind="ExternalInput")
    b1_d = nc.dram_tensor("b1", [D, 1], F32, kind="ExternalInput")
    b2_d = nc.dram_tensor("b2", [D, 1], F32, kind="ExternalInput")
    ones_d = nc.dram_tensor("ones", [1, D], BF16, kind="ExternalInput")
    outT_d = nc.dram_tensor("outT", [D, S_PAD], BF16, kind="ExternalOutput")

    # ds = exp(upd_lu/30 - now/30 + ln(1-e_lamb))
    one_m_el = max(1.0 - float(e_lamb), 1e-38)
    ds_bias = float(np.log(one_m_el) - float(now_time) / OUTPUT)
    inv_out = 1.0 / OUTPUT
    inv_lamb = 1.0 / LAMB

    with tile.TileContext(nc) as tc:
        with (
            tc.tile_pool(name="singles", bufs=1) as singles,
            tc.tile_pool(name="psm", bufs=4, space="PSUM") as psm,
            tc.tile_pool(name="dram", bufs=1, space="DRAM") as dram,
        ):
            # ---- constants ----
            ones = singles.tile([1, D], BF16)
            w1a = singles.tile([D, D], BF16)
            w1b = singles.tile([D, D], BF16)
            w2 = singles.tile([D, D], BF16)
            b1 = singles.tile([D, 1], F32)
            b2 = singles.tile([D, 1], F32)
            nc.sync.dma_start(ones, ones_d[:, :])
            nc.sync.dma_start(w1a, w1a_d[:, :])
            nc.sync.dma_start(w1b, w1b_d[:, :])
            nc.sync.dma_start(w2, w2_d[:, :])
            nc.sync.dma_start(b1, b1_d[:, :])
            nc.sync.dma_start(b2, b2_d[:, :])

            # ---- pass A: per-node scalars, pair-row layout ----
            # (own pool, released before pass B's big pools allocate)
            passa = tc.alloc_tile_pool(name="passa", bufs=1)
            lu_t = passa.tile([NP, PAIR], F32)
            ts_t = passa.tile([NEP, PAIR], F32)
            cnt_t = passa.tile([NP, PAIR], F32)
            msgc_t = passa.tile([NEP, PAIR], F32)
            nc.sync.dma_start(lu_t, lu_d[:, :])
            nc.sync.dma_start(ts_t, ts_d[:, :])
            nc.sync.dma_start(cnt_t, cnt_d[:, :])
            nc.sync.dma_start(msgc_t, msgc_d[:, :])

            dec = passa.tile([NEP, PAIR], BF16)    # event decay
            rc = passa.tile([NP, PAIR], BF16)      # 1/(cnt+eps)
            ds = passa.tile([NP, PAIR], BF16)      # (1-e_lamb)*exp((ulu-now)/30)

            # (compute-engine instructions must start at partition 0/32/64,
            #  so: full-range [0:NP) op first, then event-range [0:NEP)
            #  overwrite — both base partition 0)
            diff = passa.tile([NEP, PAIR], F32)
            nc.vector.tensor_sub(diff, lu_t[:NEP, :], ts_t[:, :])
            nc.scalar.activation(dec, diff, mybir.ActivationFunctionType.Exp,
                                 scale=inv_lamb)
            # cnt_new = cnt*decay + msgc (event region), else cnt
            cn = passa.tile([NEP, PAIR], F32)
            nc.vector.tensor_mul(cn, cnt_t[:NEP, :], dec)
            nc.vector.tensor_add(cn, cn, msgc_t[:, :])
            ce = passa.tile([NP, PAIR], F32)
            nc.vector.tensor_scalar_add(ce, cnt_t, EPS)
            nc.vector.tensor_scalar_add(ce[:NEP, :], cn, EPS)
            with nc.allow_low_precision(reason="bf16 rounding of 1/cnt"):
                nc.vector.reciprocal(rc, ce)
            # ds: event rows use ts (= updated lu), plain rows use lu
            ds_bias_t = passa.tile([NP, 1], F32)
            nc.vector.memset(ds_bias_t, ds_bias)
            nc.scalar.activation(ds, lu_t,
                                 mybir.ActivationFunctionType.Exp,
                                 scale=inv_out, bias=ds_bias_t)
            nc.scalar.activation(ds[:NEP, :], ts_t[:, :],
                                 mybir.ActivationFunctionType.Exp,
                                 scale=inv_out, bias=ds_bias_t[:NEP, :])

            # Round-trip the per-node scalars through DRAM so pass B can
            # fetch them as partition-0 rows (PE/POOL broadcast sources).
            scl = dram.tile([3, NP, PAIR], BF16)
            nc.sync.dma_start(scl[0, :, :], rc)
            nc.sync.dma_start(scl[1, :, :], ds)
            nc.sync.dma_start(scl[2, :NEP, :], dec)
            # fill the unused tail of the dec plane (chunked reads touch it)
            if NEP < NP:
                nc.sync.dma_start(scl[2, NEP:, :], rc[NEP:, :])
            passa.release()

            # ---- pass B: 5 chunks of 5 pairs of 1024 nodes ----
            io = tc.alloc_tile_pool(name="io", bufs=2)
            vrows = tc.alloc_tile_pool(name="vrows", bufs=4)
            mid = tc.alloc_tile_pool(name="mid", bufs=4)
            bc = tc.alloc_tile_pool(name="bc", bufs=4)
            for c in range(NCH):
                col0 = c * CHW
                csl = slice(col0, col0 + CHW)
                ms_ch = io.tile([D, CHW], BF16)
                nc.sync.dma_start(ms_ch, msumT_d[:, csl])
                st_ch = io.tile([D, CHW], BF16)
                nc.sync.dma_start(st_ch, staticT_d[:, csl])
                ev_pairs = max(0, min(NEP - c * CHP, CHP))  # event pairs here
                if ev_pairs > 0:
                    mg_ch = io.tile([D, CHW], BF16)
                    nc.sync.dma_start(mg_ch[:, :ev_pairs * PAIR],
                                      msgT_d[:, col0:col0 + ev_pairs * PAIR])
                out_ch = io.tile([D, CHW], BF16)

                for j in range(CHP):
                    p = c * CHP + j
                    ev = p < NEP
                    psl = slice(j * PAIR, (j + 1) * PAIR)

                    # scale rows for this pair: layout [3 rows][PAIR]
                    vch = vrows.tile([1, 3 * PAIR], BF16)
                    nc.sync.dma_start(vch, scl[:, p, :])

                    def vrow(r, half=None):
                        off = r * PAIR
                        if half is not None:
                            off += half * TILE
                            return vch[0:1, off:off + TILE]
                        return vch[0:1, off:off + PAIR]

                    # rc/ds broadcasts on GPSIMD -> SBUF bf16
                    # (bitcast to u32 to halve the element count)
                    rc_bc = bc.tile([D, PAIR], BF16, tag="rcbc")
                    nc.gpsimd.partition_broadcast(rc_bc.bitcast(U32),
                                                  vrow(0).bitcast(U32))
                    ds_bc = bc.tile([D, PAIR], BF16, tag="dsbc")
                    nc.gpsimd.partition_broadcast(ds_bc.bitcast(U32),
                                                  vrow(1).bitcast(U32))

                    if ev:
                        # decay broadcast on GPSIMD -> SBUF bf16
                        dec_b = bc.tile([D, PAIR], BF16, tag="decb")
                        nc.gpsimd.partition_broadcast(dec_b.bitcast(U32),
                                                      vrow(2).bitcast(U32))
                        m2 = mid.tile([D, PAIR], BF16)
                        nc.vector.tensor_mul(m2, ms_ch[:, psl], dec_b)
                        m3 = mid.tile([D, PAIR], BF16)
                        nc.vector.tensor_add(m3, m2, mg_ch[:, psl])
                        ftop = mid.tile([D, PAIR], BF16)
                        nc.vector.tensor_mul(ftop, m3, rc_bc)
                        fbot = m3
                    else:
                        ftop = mid.tile([D, PAIR], BF16)
                        nc.vector.tensor_mul(ftop, ms_ch[:, psl], rc_bc)
                        fbot = ms_ch[:, psl]

                    ps1 = psm.tile([D, PAIR], F32, tag="mm")
                    for h in range(2):
                        hs = slice(h * TILE, (h + 1) * TILE)
                        nc.tensor.matmul(ps1[:, hs], w1a, ftop[:, hs],
                                         start=True, stop=False)
                    for h in range(2):
                        hs = slice(h * TILE, (h + 1) * TILE)
                        nc.tensor.matmul(ps1[:, hs], w1b, fbot[:, hs],
                                         start=False, stop=True)
                    h1 = mid.tile([D, PAIR], BF16)
                    nc.scalar.activation(h1, ps1,
                                         mybir.ActivationFunctionType.Lrelu,
                                         bias=b1, scale=1.0, alpha=SLOPE)
                    ps2 = psm.tile([D, PAIR], F32, tag="mm")
                    for h in range(2):
                        hs = slice(h * TILE, (h + 1) * TILE)
                        nc.tensor.matmul(ps2[:, hs], w2, h1[:, hs],
                                         start=True, stop=True)
                    h2 = mid.tile([D, PAIR], BF16)
                    nc.scalar.activation(h2, ps2,
                                         mybir.ActivationFunctionType.Lrelu,
                                         bias=b2, scale=1.0, alpha=SLOPE)
                    t2 = mid.tile([D, PAIR], BF16)
                    nc.vector.tensor_mul(t2, h2, ds_bc)
                    nc.vector.tensor_add(out_ch[:, psl], t2, st_ch[:, psl])

                nc.sync.dma_start(outT_d[:, csl], out_ch)

            bc.release()
            mid.release()
            vrows.release()
            io.release()

    nc.compile()
    return nc


def _preprocess(memory, last_update, unique_messages, unique_timestamps,
                static_emb, W1, b1, W2, b2, e_lamb, now_time, unique_sources):
    """Shard + route events + permute; returns (in_maps, perms, NEP)."""
    memory = np.asarray(memory, dtype=np.float32)
    last_update = np.asarray(last_update, dtype=np.float32)
    unique_messages = np.asarray(unique_messages, dtype=np.float32)
    unique_timestamps = np.asarray(unique_timestamps, dtype=np.float32)
    static_emb = np.asarray(static_emb, dtype=np.float32)
    unique_sources = np.asarray(unique_sources)

    owner = unique_sources // S
    order = np.argsort(owner, kind="stable")
    counts = np.bincount(owner, minlength=NCORES)
    starts = np.concatenate([[0], np.cumsum(counts)])
    NEP = int(np.ceil(max(1, counts.max()) / PAIR))
    E_PAD = NEP * PAIR

    w1 = np.asarray(W1, dtype=np.float32)
    w1a = np.ascontiguousarray(w1[:D, :]).astype(NP_BF16)
    w1b = np.ascontiguousarray(w1[D:, :]).astype(NP_BF16)
    w2 = np.ascontiguousarray(np.asarray(W2, dtype=np.float32)).astype(NP_BF16)
    b1c = np.asarray(b1, dtype=np.float32).reshape(D, 1).copy()
    b2c = np.asarray(b2, dtype=np.float32).reshape(D, 1).copy()
    ones = np.ones((1, D), dtype=NP_BF16)

    in_maps = []
    perms = []
    for c in range(NCORES):
        ev_rows = order[starts[c]:starts[c + 1]]
        src_local = unique_sources[ev_rows] - c * S
        E_c = src_local.shape[0]

        is_ev = np.zeros(S, dtype=bool)
        is_ev[src_local] = True
        non_ev = np.nonzero(~is_ev)[0]
        perm = np.concatenate([src_local, non_ev]).astype(np.int64)
        perms.append(perm)

        mem_pad = np.empty((S_PAD, D + 1), dtype=np.float32)
        mem_pad[:S] = memory[c * S:(c + 1) * S][perm]
        mem_pad[S:, :D] = 0.0
        mem_pad[S:, D] = 1.0
        lu_pad = np.zeros(S_PAD, dtype=np.float32)
        lu_pad[:S] = last_update[c * S:(c + 1) * S][perm]
        st_pad = np.zeros((S_PAD, D), dtype=np.float32)
        st_pad[:S] = static_emb[c * S:(c + 1) * S][perm]
        st_pad *= np.float32(e_lamb)   # fold e_lamb into the static table

        msg_full = np.zeros((E_PAD, D + 1), dtype=np.float32)
        msg_full[:E_c] = unique_messages[ev_rows]
        ts_full = np.empty(E_PAD, dtype=np.float32)
        ts_full[:E_c] = unique_timestamps[ev_rows]
        ts_full[E_c:] = lu_pad[E_c:E_PAD]   # identity events: ts = lu, msg = 0

        in_maps.append({
            "msumT": np.ascontiguousarray(mem_pad[:, :D].T).astype(NP_BF16),
            "staticT": np.ascontiguousarray(st_pad.T).astype(NP_BF16),
            "msgT": np.ascontiguousarray(msg_full[:, :D].T).astype(NP_BF16),
            "lu_t": lu_pad.reshape(NP, PAIR).copy(),
            "ts_t": ts_full.reshape(NEP, PAIR).copy(),
            "cnt_t": mem_pad[:, D].reshape(NP, PAIR).copy(),
            "msgc_t": msg_full[:, D].reshape(NEP, PAIR).copy(),
            "w1a": w1a, "w1b": w1b, "w2": w2,
            "b1": b1c, "b2": b2c, "ones": ones,
        })
    return in_maps, perms, NEP


def _run(inputs, trace=False, trace_cores=None):
    in_maps, perms, NEP = _preprocess(**inputs)
    nc = _build(NEP, inputs["e_lamb"], inputs["now_time"])
    res = run_bass_kernel_spmd(nc, in_maps, core_ids=list(range(NCORES)),
                               trace=trace, trace_cores=trace_cores)
    out = np.empty((N_NODES, D), dtype=np.float32)
    for c in range(NCORES):
        out_perm = res.results[c]["outT"].T[:S].astype(np.float32)
        shard = np.empty((S, D), dtype=np.float32)
        shard[perms[c]] = out_perm
        out[c * S:(c + 1) * S] = shard
    return out, res


def kernel(**inputs) -> np.ndarray:
    out, _ = _run(inputs, trace=False)
    return out


# revision 25
# speedup vs baseline: 1.0236x; 1.0236x over previous
"""CTDG encoder (exp-decay memory GNN) on 8 Trainium2 NeuronCores.

Strategy (pure node-parallel, per the natural sharding of this module):
- Host: shard the 200k nodes into 8 contiguous ranges of 25000 (padded to
  25600 = 25*1024), route each event (unique_sources row) to its owning
  shard, and permute each shard so event nodes come first.  The event
  region is padded to a uniform multiple of 1024 with identity events
  (msg=0, ts=last_update), so every 1024-node pair of device tiles is
  either fully "event" or fully "plain".  memory/static_emb/messages are
  pre-transposed to feature-major [128, nodes] (bf16) so the device never
  transposes.
- Device (SPMD, identical program, per-core data):
  Pass A: per-node scalars in pair-row layout [25, 1024] (f32 math):
      decay = exp((lu - ts)/30), rc = 1/(cnt_new + eps),
      ds = (1 - e_lamb) * exp((upd_lu - now)/30)   (as exp(x/30 + bias))
    then round-tripped through DRAM (bf16) so pass B can fetch them as
    partition-0 rows.
  Pass B: for each of 25 pairs (1024 nodes):
      rc/ds broadcast to [128,1024] SBUF via GPSIMD partition_broadcast
      (uint32-bitcast to halve element count), decay broadcast via two
      K=1 bf16 matmuls (PE), event update + count-normalize + output
      combine on DVE (bf16 2x, 1024-wide), two-layer MLP on PE (bf16,
      512-wide into paired PSUM banks), LeakyReLU (+bias) on ACT
      (1024-wide), IO in 5-pair chunked DMAs.
- Host: inverse-permute, upcast, and concatenate shard outputs.
"""

import numpy as np
import ml_dtypes

import concourse.bacc as bacc
import concourse.tile as tile
from concourse import mybir
from concourse.bass_utils import run_bass_kernel_spmd

N_NODES = 200000
D = 128
NCORES = 8
S = N_NODES // NCORES          # 25000 real nodes per core
TILE = 512                     # matmul / PSUM-bank granularity
PAIR = 1024                    # elementwise granularity
NP = 25                        # pairs per core
S_PAD = NP * PAIR              # 25600
CHP = 5                        # pairs per IO chunk
NCH = NP // CHP                # 5 chunks
CHW = CHP * PAIR               # 5120 columns per chunk
LAMB = 30.0                    # memory-updater decay constant
OUTPUT = 30.0                  # embedding time-decay constant
EPS = 1e-10
SLOPE = 0.01

F32 = mybir.dt.float32
BF16 = mybir.dt.bfloat16
U32 = mybir.dt.uint32
NP_BF16 = ml_dtypes.bfloat16


def _build(NEP, e_lamb, now_time):
    """Build the per-core bass program. NEP = number of event pairs."""
    nc = bacc.Bacc("TRN2", target_bir_lowering=False, debug=False,
                   num_devices=NCORES)
    E_PAD = NEP * PAIR

    msumT_d = nc.dram_tensor("msumT", [D, S_PAD], BF16, kind="ExternalInput")
    # staticT is pre-scaled by e_lamb on the host (constant folding)
    staticT_d = nc.dram_tensor("staticT", [D, S_PAD], BF16, kind="ExternalInput")
    msgT_d = nc.dram_tensor("msgT", [D, E_PAD], BF16, kind="ExternalInput")
    lu_d = nc.dram_tensor("lu_t", [NP, PAIR], F32, kind="ExternalInput")
    ts_d = nc.dram_tensor("ts_t", [NEP, PAIR], F32, kind="ExternalInput")
    cnt_d = nc.dram_tensor("cnt_t", [NP, PAIR], F32, kind="ExternalInput")
    msgc_d = nc.dram_tensor("msgc_t", [NEP, PAIR], F32, kind="ExternalInput")
    w1a_d = nc.dram_tensor("w1a", [D, D], BF16, kind="ExternalInput")
    w1b_d = nc.dram_tensor("w1b", [D, D], BF16, kind="ExternalInput")
    w2_d = nc.dram_tensor("w2", [D, D], BF16, kind="ExternalInput")
    b1_d = nc.dram_tensor("b1", [D, 1], F32, kind="ExternalInput")
    b2_d = nc.dram_tensor("b2", [D, 1], F32, kind="ExternalInput")
    ones_d = nc.dram_tensor("ones", [1, D], BF16, kind="ExternalInput")
    outT_d = nc.dram_tensor("outT", [D, S_PAD], BF16, kind="ExternalOutput")

    # ds = exp(upd_lu/30 - now/30 + ln(1-e_lamb))
    one_m_el = max(1.0 - float(e_lamb), 1e-38)
    ds_bias = float(np.log(one_m_el) - float(now_time) / OUTPUT)
    inv_out = 1.0 / OUTPUT
    inv_lamb = 1.0 / LAMB

    with tile.TileContext(nc) as tc:
        with (
            tc.tile_pool(name="singles", bufs=1) as singles,
            tc.tile_pool(name="psm", bufs=4, space="PSUM") as psm,
            tc.tile_pool(name="dram", bufs=1, space="DRAM") as dram,
        ):
            # ---- constants ----
            ones = singles.tile([1, D], BF16)
            w1a = singles.tile([D, D], BF16)
            w1b = singles.tile([D, D], BF16)
            w2 = singles.tile([D, D], BF16)
            b1 = singles.tile([D, 1], F32)
            b2 = singles.tile([D, 1], F32)
            nc.sync.dma_start(ones, ones_d[:, :])
            nc.sync.dma_start(w1a, w1a_d[:, :])
            nc.sync.dma_start(w1b, w1b_d[:, :])
            nc.sync.dma_start(w2, w2_d[:, :])
            nc.sync.dma_start(b1, b1_d[:, :])
            nc.sync.dma_start(b2, b2_d[:, :])

            # pass-A outputs live in a persistent pool: the scl writes read
            # them after passa's address space is already recycled.
            res = tc.alloc_tile_pool(name="res", bufs=1)

            # ---- pass A: per-node scalars, pair-row layout ----
            # (own pool, released before pass B's big pools allocate)
            passa = tc.alloc_tile_pool(name="passa", bufs=1)
            lu_t = passa.tile([NP, PAIR], F32)
            ts_t = passa.tile([NEP, PAIR], F32)
            cnt_t = passa.tile([NP, PAIR], F32)
            msgc_t = passa.tile([NEP, PAIR], F32)
            nc.sync.dma_start(lu_t, lu_d[:, :])
            nc.sync.dma_start(ts_t, ts_d[:, :])
            nc.sync.dma_start(cnt_t, cnt_d[:, :])
            nc.sync.dma_start(msgc_t, msgc_d[:, :])

            # pass-A outputs live in the persistent pool: the rearrange DMAs
            # read them after passa's address space is already recycled
            dec = res.tile([NEP, PAIR], BF16)      # event decay
            rc = res.tile([NP, PAIR], BF16)        # 1/(cnt+eps)
            ds = res.tile([NP, PAIR], BF16)        # (1-e_lamb)*exp((ulu-now)/30)

            # (compute-engine instructions must start at partition 0/32/64,
            #  so: full-range [0:NP) op first, then event-range [0:NEP)
            #  overwrite — both base partition 0)
            diff = passa.tile([NEP, PAIR], F32)
            nc.vector.tensor_sub(diff, lu_t[:NEP, :], ts_t[:, :])
            nc.scalar.activation(dec, diff, mybir.ActivationFunctionType.Exp,
                                 scale=inv_lamb)
            # cnt_new = cnt*decay + msgc (event region), else cnt
            cn = passa.tile([NEP, PAIR], F32)
            nc.vector.tensor_mul(cn, cnt_t[:NEP, :], dec)
            nc.vector.tensor_add(cn, cn, msgc_t[:, :])
            ce = passa.tile([NP, PAIR], F32)
            nc.vector.tensor_scalar_add(ce, cnt_t, EPS)
            nc.vector.tensor_scalar_add(ce[:NEP, :], cn, EPS)
            with nc.allow_low_precision(reason="bf16 rounding of 1/cnt"):
                nc.vector.reciprocal(rc, ce)
            # ds: event rows use ts (= updated lu), plain rows use lu
            ds_bias_t = passa.tile([NP, 1], F32)
            nc.vector.memset(ds_bias_t, ds_bias)
            nc.scalar.activation(ds, lu_t,
                                 mybir.ActivationFunctionType.Exp,
                                 scale=inv_out, bias=ds_bias_t)
            nc.scalar.activation(ds[:NEP, :], ts_t[:, :],
                                 mybir.ActivationFunctionType.Exp,
                                 scale=inv_out, bias=ds_bias_t[:NEP, :])

            # Park the per-node scalars in DRAM; pass B fetches each pair's
            # three rows as one partition-0 row.  The writes go on the
            # *scalar* queue so the sync queue's chunk loads never wait
            # behind pass A.
            scl = dram.tile([3, NP, PAIR], BF16)
            nc.scalar.dma_start(scl[0, :, :], rc)
            nc.scalar.dma_start(scl[1, :, :], ds)
            nc.scalar.dma_start(scl[2, :NEP, :], dec)
            if NEP < NP:
                nc.scalar.dma_start(scl[2, NEP:, :], rc[NEP:, :])
            passa.release()

            # ---- pass B: 5 chunks of 5 pairs of 1024 nodes ----
            io = tc.alloc_tile_pool(name="io", bufs=2)
            vrows = tc.alloc_tile_pool(name="vrows", bufs=6)
            mid = tc.alloc_tile_pool(name="mid", bufs=3)
            bc = tc.alloc_tile_pool(name="bc", bufs=4)
            for c in range(NCH):
                col0 = c * CHW
                csl = slice(col0, col0 + CHW)
                ms_ch = io.tile([D, CHW], BF16)
                nc.sync.dma_start(ms_ch, msumT_d[:, csl])
                st_ch = io.tile([D, CHW], BF16)
                nc.sync.dma_start(st_ch, staticT_d[:, csl])
                ev_pairs = max(0, min(NEP - c * CHP, CHP))  # event pairs here
                if ev_pairs > 0:
                    mg_ch = io.tile([D, CHW], BF16)
                    nc.sync.dma_start(mg_ch[:, :ev_pairs * PAIR],
                                      msgT_d[:, col0:col0 + ev_pairs * PAIR])
                out_ch = io.tile([D, CHW], BF16)

                for j in range(CHP):
                    p = c * CHP + j
                    ev = p < NEP
                    psl = slice(j * PAIR, (j + 1) * PAIR)

                    # scale rows for this pair: layout [3 rows][PAIR]
                    vch = vrows.tile([1, 3 * PAIR], BF16)
                    nc.sync.dma_start(vch, scl[:, p, :])

                    def vrow(r):
                        off = r * PAIR
                        return vch[0:1, off:off + PAIR]

                    # rc/ds broadcasts on GPSIMD -> SBUF bf16
                    # (bitcast to u32 to halve the element count)
                    rc_bc = bc.tile([D, PAIR], BF16, tag="rcbc")
                    nc.gpsimd.partition_broadcast(rc_bc.bitcast(U32),
                                                  vrow(0).bitcast(U32))
                    ds_bc = bc.tile([D, PAIR], BF16, tag="dsbc")
                    nc.gpsimd.partition_broadcast(ds_bc.bitcast(U32),
                                                  vrow(1).bitcast(U32))

                    if ev:
                        # decay broadcast on GPSIMD -> SBUF bf16
                        dec_b = bc.tile([D, PAIR], BF16, tag="decb")
                        nc.gpsimd.partition_broadcast(dec_b.bitcast(U32),
                                                      vrow(2).bitcast(U32))
                        m2 = mid.tile([D, PAIR], BF16)
                        nc.vector.tensor_mul(m2, ms_ch[:, psl], dec_b)
                        m3 = mid.tile([D, PAIR], BF16)
                        nc.vector.tensor_add(m3, m2, mg_ch[:, psl])
                        ftop = mid.tile([D, PAIR], BF16)
                        nc.vector.tensor_mul(ftop, m3, rc_bc)
                        fbot = m3
                    else:
                        ftop = mid.tile([D, PAIR], BF16)
                        nc.vector.tensor_mul(ftop, ms_ch[:, psl], rc_bc)
                        fbot = ms_ch[:, psl]

                    ps1 = psm.tile([D, PAIR], F32, tag="mm")
                    for h in range(2):
                        hs = slice(h * TILE, (h + 1) * TILE)
                        nc.tensor.matmul(ps1[:, hs], w1a, ftop[:, hs],
                                         start=True, stop=False)
                    for h in range(2):
                        hs = slice(h * TILE, (h + 1) * TILE)
                        nc.tensor.matmul(ps1[:, hs], w1b, fbot[:, hs],
                                         start=False, stop=True)
                    h1 = mid.tile([D, PAIR], BF16)
                    nc.scalar.activation(h1, ps1,
                                         mybir.ActivationFunctionType.Lrelu,
                                         bias=b1, scale=1.0, alpha=SLOPE)
                    ps2 = psm.tile([D, PAIR], F32, tag="mm")
                    for h in range(2):
                        hs = slice(h * TILE, (h + 1) * TILE)
                        nc.tensor.matmul(ps2[:, hs], w2, h1[:, hs],
                                         start=True, stop=True)
                    h2 = mid.tile([D, PAIR], BF16)
                    nc.scalar.activation(h2, ps2,
                                         mybir.ActivationFunctionType.Lrelu,
                                         bias=b2, scale=1.0, alpha=SLOPE)
                    t2 = mid.tile([D, PAIR], BF16)
                    nc.vector.tensor_mul(t2, h2, ds_bc)
                    nc.vector.tensor_add(out_ch[:, psl], t2, st_ch[:, psl])

                nc.sync.dma_start(outT_d[:, csl], out_ch)

            bc.release()
            mid.release()
            vrows.release()
            io.release()
            res.release()

    nc.compile()
    return nc


def _preprocess(memory, last_update, unique_messages, unique_timestamps,
                static_emb, W1, b1, W2, b2, e_lamb, now_time, unique_sources):
    """Shard + route events + permute; returns (in_maps, perms, NEP)."""
    memory = np.asarray(memory, dtype=np.float32)
    last_update = np.asarray(last_update, dtype=np.float32)
    unique_messages = np.asarray(unique_messages, dtype=np.float32)
    unique_timestamps = np.asarray(unique_timestamps, dtype=np.float32)
    static_emb = np.asarray(static_emb, dtype=np.float32)
    unique_sources = np.asarray(unique_sources)

    owner = unique_sources // S
    order = np.argsort(owner, kind="stable")
    counts = np.bincount(owner, minlength=NCORES)
    starts = np.concatenate([[0], np.cumsum(counts)])
    NEP = int(np.ceil(max(1, counts.max()) / PAIR))
    E_PAD = NEP * PAIR

    w1 = np.asarray(W1, dtype=np.float32)
    w1a = np.ascontiguousarray(w1[:D, :]).astype(NP_BF16)
    w1b = np.ascontiguousarray(w1[D:, :]).astype(NP_BF16)
    w2 = np.ascontiguousarray(np.asarray(W2, dtype=np.float32)).astype(NP_BF16)
    b1c = np.asarray(b1, dtype=np.float32).reshape(D, 1).copy()
    b2c = np.asarray(b2, dtype=np.float32).reshape(D, 1).copy()
    ones = np.ones((1, D), dtype=NP_BF16)

    in_maps = []
    perms = []
    for c in range(NCORES):
        ev_rows = order[starts[c]:starts[c + 1]]
        src_local = unique_sources[ev_rows] - c * S
        E_c = src_local.shape[0]

        is_ev = np.zeros(S, dtype=bool)
        is_ev[src_local] = True
        non_ev = np.nonzero(~is_ev)[0]
        perm = np.concatenate([src_local, non_ev]).astype(np.int64)
        perms.append(perm)

        mem_pad = np.empty((S_PAD, D + 1), dtype=np.float32)
        mem_pad[:S] = memory[c * S:(c + 1) * S][perm]
        mem_pad[S:, :D] = 0.0
        mem_pad[S:, D] = 1.0
        lu_pad = np.zeros(S_PAD, dtype=np.float32)
        lu_pad[:S] = last_update[c * S:(c + 1) * S][perm]
        st_pad = np.zeros((S_PAD, D), dtype=np.float32)
        st_pad[:S] = static_emb[c * S:(c + 1) * S][perm]
        st_pad *= np.float32(e_lamb)   # fold e_lamb into the static table

        msg_full = np.zeros((E_PAD, D + 1), dtype=np.float32)
        msg_full[:E_c] = unique_messages[ev_rows]
        ts_full = np.empty(E_PAD, dtype=np.float32)
        ts_full[:E_c] = unique_timestamps[ev_rows]
        ts_full[E_c:] = lu_pad[E_c:E_PAD]   # identity events: ts = lu, msg = 0

        in_maps.append({
            "msumT": np.ascontiguousarray(mem_pad[:, :D].T).astype(NP_BF16),
            "staticT": np.ascontiguousarray(st_pad.T).astype(NP_BF16),
            "msgT": np.ascontiguousarray(msg_full[:, :D].T).astype(NP_BF16),
            "lu_t": lu_pad.reshape(NP, PAIR).copy(),
            "ts_t": ts_full.reshape(NEP, PAIR).copy(),
            "cnt_t": mem_pad[:, D].reshape(NP, PAIR).copy(),
            "msgc_t": msg_full[:, D].reshape(NEP, PAIR).copy(),
            "w1a": w1a, "w1b": w1b, "w2": w2,
            "b1": b1c, "b2": b2c, "ones": ones,
        })
    return in_maps, perms, NEP


def _run(inputs, trace=False, trace_cores=None):
    in_maps, perms, NEP = _preprocess(**inputs)
    nc = _build(NEP, inputs["e_lamb"], inputs["now_time"])
    res = run_bass_kernel_spmd(nc, in_maps, core_ids=list(range(NCORES)),
                               trace=trace, trace_cores=trace_cores)
    out = np.empty((N_NODES, D), dtype=np.float32)
    for c in range(NCORES):
        out_perm = res.results[c]["outT"].T[:S].astype(np.float32)
        shard = np.empty((S, D), dtype=np.float32)
        shard[perms[c]] = out_perm
        out[c * S:(c + 1) * S] = shard
    return out, res


def kernel(**inputs) -> np.ndarray:
    out, _ = _run(inputs, trace=False)
    return out


# revision 26
# speedup vs baseline: 1.2691x; 1.2398x over previous
"""CTDG encoder (exp-decay memory GNN) on 8 Trainium2 NeuronCores.

Strategy (pure node-parallel, per the natural sharding of this module):
- Host: shard the 200k nodes into 8 contiguous ranges of 25000 (padded to
  25600 = 25*1024), route each event (unique_sources row) to its owning
  shard, and permute each shard so event nodes come first.  The event
  region is padded to a uniform multiple of 1024 with identity events
  (msg=0, ts=last_update), so every 1024-node pair of device tiles is
  either fully "event" or fully "plain".  memory/static_emb/messages are
  pre-transposed to feature-major [128, nodes] (bf16) so the device never
  transposes.
- Device (SPMD, identical program, per-core data):
  Pass A: per-node scalars in pair-row layout [25, 1024] (f32 math):
      decay = exp((lu - ts)/30), rc = 1/(cnt_new + eps),
      ds = (1 - e_lamb) * exp((upd_lu - now)/30)   (as exp(x/30 + bias))
    then round-tripped through DRAM (bf16) so pass B can fetch them as
    partition-0 rows.
  Pass B: for each of 25 pairs (1024 nodes):
      rc/ds broadcast to [128,1024] SBUF via GPSIMD partition_broadcast
      (uint32-bitcast to halve element count), decay broadcast via two
      K=1 bf16 matmuls (PE), event update + count-normalize + output
      combine on DVE (bf16 2x, 1024-wide), two-layer MLP on PE (bf16,
      512-wide into paired PSUM banks), LeakyReLU (+bias) on ACT
      (1024-wide), IO in 5-pair chunked DMAs.
- Host: inverse-permute, upcast, and concatenate shard outputs.
"""

import numpy as np
import ml_dtypes

import concourse.bacc as bacc
import concourse.tile as tile
from concourse import mybir
from concourse.bass_utils import run_bass_kernel_spmd

N_NODES = 200000
D = 128
NCORES = 8
S = N_NODES // NCORES          # 25000 real nodes per core
TILE = 512                     # matmul / PSUM-bank granularity
PAIR = 1024                    # elementwise granularity
NP = 25                        # pairs per core
S_PAD = NP * PAIR              # 25600
CHP = 5                        # pairs per IO chunk
NCH = NP // CHP                # 5 chunks
CHW = CHP * PAIR               # 5120 columns per chunk
LAMB = 30.0                    # memory-updater decay constant
OUTPUT = 30.0                  # embedding time-decay constant
EPS = 1e-10
SLOPE = 0.01

F32 = mybir.dt.float32
BF16 = mybir.dt.bfloat16
U32 = mybir.dt.uint32
NP_BF16 = ml_dtypes.bfloat16


def _build(NEP, e_lamb, now_time):
    """Build the per-core bass program. NEP = number of event pairs."""
    nc = bacc.Bacc("TRN2", target_bir_lowering=False, debug=False,
                   num_devices=NCORES)
    E_PAD = NEP * PAIR

    msumT_d = nc.dram_tensor("msumT", [D, S_PAD], BF16, kind="ExternalInput")
    # staticT is pre-scaled by e_lamb on the host (constant folding)
    staticT_d = nc.dram_tensor("staticT", [D, S_PAD], BF16, kind="ExternalInput")
    msgT_d = nc.dram_tensor("msgT", [D, E_PAD], BF16, kind="ExternalInput")
    lu_d = nc.dram_tensor("lu_t", [NP, PAIR], F32, kind="ExternalInput")
    ts_d = nc.dram_tensor("ts_t", [NEP, PAIR], F32, kind="ExternalInput")
    cnt_d = nc.dram_tensor("cnt_t", [NP, PAIR], F32, kind="ExternalInput")
    msgc_d = nc.dram_tensor("msgc_t", [NEP, PAIR], F32, kind="ExternalInput")
    w1a_d = nc.dram_tensor("w1a", [D, D], BF16, kind="ExternalInput")
    w1b_d = nc.dram_tensor("w1b", [D, D], BF16, kind="ExternalInput")
    w2_d = nc.dram_tensor("w2", [D, D], BF16, kind="ExternalInput")
    b1_d = nc.dram_tensor("b1", [D, 1], F32, kind="ExternalInput")
    b2_d = nc.dram_tensor("b2", [D, 1], F32, kind="ExternalInput")
    ones_d = nc.dram_tensor("ones", [1, D], BF16, kind="ExternalInput")
    outT_d = nc.dram_tensor("outT", [D, S_PAD], BF16, kind="ExternalOutput")

    # ds = exp(upd_lu/30 - now/30 + ln(1-e_lamb))
    one_m_el = max(1.0 - float(e_lamb), 1e-38)
    ds_bias = float(np.log(one_m_el) - float(now_time) / OUTPUT)
    inv_out = 1.0 / OUTPUT
    inv_lamb = 1.0 / LAMB

    with tile.TileContext(nc) as tc:
        with (
            tc.tile_pool(name="singles", bufs=1) as singles,
            tc.tile_pool(name="psm", bufs=4, space="PSUM") as psm,
            tc.tile_pool(name="dram", bufs=1, space="DRAM") as dram,
        ):
            # ---- constants ----
            ones = singles.tile([1, D], BF16)
            w1a = singles.tile([D, D], BF16)
            w1b = singles.tile([D, D], BF16)
            w2 = singles.tile([D, D], BF16)
            b1 = singles.tile([D, 1], F32)
            b2 = singles.tile([D, 1], F32)

            # pass-A outputs live in a persistent pool: the scl writes read
            # them after passa's address space is already recycled.
            res = tc.alloc_tile_pool(name="res", bufs=1)

            # ---- pass A: per-node scalars, pair-row layout ----
            # (own pool, released before pass B's big pools allocate)
            passa = tc.alloc_tile_pool(name="passa", bufs=1)
            lu_t = passa.tile([NP, PAIR], F32)
            ts_t = passa.tile([NEP, PAIR], F32)
            cnt_t = passa.tile([NP, PAIR], F32)
            msgc_t = passa.tile([NEP, PAIR], F32)
            nc.sync.dma_start(lu_t, lu_d[:, :])
            nc.sync.dma_start(ts_t, ts_d[:, :])
            nc.sync.dma_start(cnt_t, cnt_d[:, :])
            nc.sync.dma_start(msgc_t, msgc_d[:, :])
            nc.sync.dma_start(ones, ones_d[:, :])
            nc.sync.dma_start(w1a, w1a_d[:, :])
            nc.sync.dma_start(w1b, w1b_d[:, :])
            nc.sync.dma_start(w2, w2_d[:, :])
            nc.sync.dma_start(b1, b1_d[:, :])
            nc.sync.dma_start(b2, b2_d[:, :])

            # pass-A outputs live in the persistent pool: the rearrange DMAs
            # read them after passa's address space is already recycled
            dec = res.tile([NEP, PAIR], BF16)      # event decay
            rc = res.tile([NP, PAIR], BF16)        # 1/(cnt+eps)
            ds = res.tile([NP, PAIR], BF16)        # (1-e_lamb)*exp((ulu-now)/30)

            # (compute-engine instructions must start at partition 0/32/64,
            #  so: full-range [0:NP) op first, then event-range [0:NEP)
            #  overwrite — both base partition 0)
            diff = passa.tile([NEP, PAIR], F32)
            nc.vector.tensor_sub(diff, lu_t[:NEP, :], ts_t[:, :])
            nc.scalar.activation(dec, diff, mybir.ActivationFunctionType.Exp,
                                 scale=inv_lamb)
            # cnt_new = cnt*decay + msgc (event region), else cnt
            cn = passa.tile([NEP, PAIR], F32)
            nc.vector.tensor_mul(cn, cnt_t[:NEP, :], dec)
            nc.vector.tensor_add(cn, cn, msgc_t[:, :])
            ce = passa.tile([NP, PAIR], F32)
            nc.vector.tensor_scalar_add(ce, cnt_t, EPS)
            nc.vector.tensor_scalar_add(ce[:NEP, :], cn, EPS)
            rcf = passa.tile([NP, PAIR], F32)
            nc.vector.reciprocal_approx_fast(rcf, ce)   # 18 bits; rc is bf16
            with nc.allow_low_precision(reason="bf16 rounding of 1/cnt"):
                nc.vector.tensor_copy(rc, rcf)
            # ds: event rows use ts (= updated lu), plain rows use lu
            ds_bias_t = passa.tile([NP, 1], F32)
            nc.vector.memset(ds_bias_t, ds_bias)
            nc.scalar.activation(ds, lu_t,
                                 mybir.ActivationFunctionType.Exp,
                                 scale=inv_out, bias=ds_bias_t)
            nc.scalar.activation(ds[:NEP, :], ts_t[:, :],
                                 mybir.ActivationFunctionType.Exp,
                                 scale=inv_out, bias=ds_bias_t[:NEP, :])

            # Park the per-node scalars in DRAM; pass B fetches each pair's
            # three rows as one partition-0 row.  The writes go on the
            # *scalar* queue so the sync queue's chunk loads never wait
            # behind pass A.
            scl = dram.tile([3, NP, PAIR], BF16)
            nc.scalar.dma_start(scl[0, :, :], rc)
            nc.scalar.dma_start(scl[1, :, :], ds)
            nc.scalar.dma_start(scl[2, :NEP, :], dec)
            if NEP < NP:
                nc.scalar.dma_start(scl[2, NEP:, :], rc[NEP:, :])
            passa.release()

            # ---- pass B: 5 chunks of 5 pairs of 1024 nodes ----
            io = tc.alloc_tile_pool(name="io", bufs=2)
            vrows = tc.alloc_tile_pool(name="vrows", bufs=6)
            mid = tc.alloc_tile_pool(name="mid", bufs=3)
            bc = tc.alloc_tile_pool(name="bc", bufs=4)
            for c in range(NCH):
                col0 = c * CHW
                csl = slice(col0, col0 + CHW)
                ms_ch = io.tile([D, CHW], BF16)
                nc.sync.dma_start(ms_ch, msumT_d[:, csl])
                st_ch = io.tile([D, CHW], BF16)
                nc.sync.dma_start(st_ch, staticT_d[:, csl])
                ev_pairs = max(0, min(NEP - c * CHP, CHP))  # event pairs here
                if ev_pairs > 0:
                    mg_ch = io.tile([D, CHW], BF16)
                    nc.sync.dma_start(mg_ch[:, :ev_pairs * PAIR],
                                      msgT_d[:, col0:col0 + ev_pairs * PAIR])
                out_ch = io.tile([D, CHW], BF16)

                for j in range(CHP):
                    p = c * CHP + j
                    ev = p < NEP
                    psl = slice(j * PAIR, (j + 1) * PAIR)

                    # scale rows for this pair: layout [3 rows][PAIR]
                    vch = vrows.tile([1, 3 * PAIR], BF16)
                    nc.scalar.dma_start(vch, scl[:, p, :])

                    def vrow(r):
                        off = r * PAIR
                        return vch[0:1, off:off + PAIR]

                    # rc/ds broadcasts on GPSIMD -> SBUF bf16
                    # (bitcast to u32 to halve the element count)
                    rc_bc = bc.tile([D, PAIR], BF16, tag="rcbc")
                    nc.gpsimd.partition_broadcast(rc_bc.bitcast(U32),
                                                  vrow(0).bitcast(U32))
                    ds_bc = bc.tile([D, PAIR], BF16, tag="dsbc")
                    nc.gpsimd.partition_broadcast(ds_bc.bitcast(U32),
                                                  vrow(1).bitcast(U32))

                    if ev:
                        # decay broadcast on GPSIMD -> SBUF bf16
                        dec_b = bc.tile([D, PAIR], BF16, tag="decb")
                        nc.gpsimd.partition_broadcast(dec_b.bitcast(U32),
                                                      vrow(2).bitcast(U32))
                        m2 = mid.tile([D, PAIR], BF16)
                        nc.vector.tensor_mul(m2, ms_ch[:, psl], dec_b)
                        m3 = mid.tile([D, PAIR], BF16)
                        nc.vector.tensor_add(m3, m2, mg_ch[:, psl])
                        ftop = mid.tile([D, PAIR], BF16)
                        nc.vector.tensor_mul(ftop, m3, rc_bc)
                        fbot = m3
                    else:
                        ftop = mid.tile([D, PAIR], BF16)
                        nc.vector.tensor_mul(ftop, ms_ch[:, psl], rc_bc)
                        fbot = ms_ch[:, psl]

                    ps1 = psm.tile([D, PAIR], F32, tag="mm")
                    for h in range(2):
                        hs = slice(h * TILE, (h + 1) * TILE)
                        nc.tensor.matmul(ps1[:, hs], w1a, ftop[:, hs],
                                         start=True, stop=False)
                    for h in range(2):
                        hs = slice(h * TILE, (h + 1) * TILE)
                        nc.tensor.matmul(ps1[:, hs], w1b, fbot[:, hs],
                                         start=False, stop=True)
                    h1 = mid.tile([D, PAIR], BF16)
                    nc.scalar.activation(h1, ps1,
                                         mybir.ActivationFunctionType.Lrelu,
                                         bias=b1, scale=1.0, alpha=SLOPE)
                    ps2 = psm.tile([D, PAIR], F32, tag="mm")
                    for h in range(2):
                        hs = slice(h * TILE, (h + 1) * TILE)
                        nc.tensor.matmul(ps2[:, hs], w2, h1[:, hs],
                                         start=True, stop=True)
                    h2 = mid.tile([D, PAIR], BF16)
                    nc.scalar.activation(h2, ps2,
                                         mybir.ActivationFunctionType.Lrelu,
                                         bias=b2, scale=1.0, alpha=SLOPE)
                    t2 = mid.tile([D, PAIR], BF16)
                    nc.vector.tensor_mul(t2, h2, ds_bc)
                    nc.vector.tensor_add(out_ch[:, psl], t2, st_ch[:, psl])

                nc.sync.dma_start(outT_d[:, csl], out_ch)

            bc.release()
            mid.release()
            vrows.release()
            io.release()
            res.release()

    nc.compile()
    return nc


def _preprocess(memory, last_update, unique_messages, unique_timestamps,
                static_emb, W1, b1, W2, b2, e_lamb, now_time, unique_sources):
    """Shard + route events + permute; returns (in_maps, perms, NEP)."""
    memory = np.asarray(memory, dtype=np.float32)
    last_update = np.asarray(last_update, dtype=np.float32)
    unique_messages = np.asarray(unique_messages, dtype=np.float32)
    unique_timestamps = np.asarray(unique_timestamps, dtype=np.float32)
    static_emb = np.asarray(static_emb, dtype=np.float32)
    unique_sources = np.asarray(unique_sources)

    owner = unique_sources // S
    order = np.argsort(owner, kind="stable")
    counts = np.bincount(owner, minlength=NCORES)
    starts = np.concatenate([[0], np.cumsum(counts)])
    NEP = int(np.ceil(max(1, counts.max()) / PAIR))
    E_PAD = NEP * PAIR

    w1 = np.asarray(W1, dtype=np.float32)
    w1a = np.ascontiguousarray(w1[:D, :]).astype(NP_BF16)
    w1b = np.ascontiguousarray(w1[D:, :]).astype(NP_BF16)
    w2 = np.ascontiguousarray(np.asarray(W2, dtype=np.float32)).astype(NP_BF16)
    b1c = np.asarray(b1, dtype=np.float32).reshape(D, 1).copy()
    b2c = np.asarray(b2, dtype=np.float32).reshape(D, 1).copy()
    ones = np.ones((1, D), dtype=NP_BF16)

    in_maps = []
    perms = []
    for c in range(NCORES):
        ev_rows = order[starts[c]:starts[c + 1]]
        src_local = unique_sources[ev_rows] - c * S
        E_c = src_local.shape[0]

        is_ev = np.zeros(S, dtype=bool)
        is_ev[src_local] = True
        non_ev = np.nonzero(~is_ev)[0]
        perm = np.concatenate([src_local, non_ev]).astype(np.int64)
        perms.append(perm)

        mem_pad = np.empty((S_PAD, D + 1), dtype=np.float32)
        mem_pad[:S] = memory[c * S:(c + 1) * S][perm]
        mem_pad[S:, :D] = 0.0
        mem_pad[S:, D] = 1.0
        lu_pad = np.zeros(S_PAD, dtype=np.float32)
        lu_pad[:S] = last_update[c * S:(c + 1) * S][perm]
        st_pad = np.zeros((S_PAD, D), dtype=np.float32)
        st_pad[:S] = static_emb[c * S:(c + 1) * S][perm]
        st_pad *= np.float32(e_lamb)   # fold e_lamb into the static table

        msg_full = np.zeros((E_PAD, D + 1), dtype=np.float32)
        msg_full[:E_c] = unique_messages[ev_rows]
        ts_full = np.empty(E_PAD, dtype=np.float32)
        ts_full[:E_c] = unique_timestamps[ev_rows]
        ts_full[E_c:] = lu_pad[E_c:E_PAD]   # identity events: ts = lu, msg = 0

        in_maps.append({
            "msumT": np.ascontiguousarray(mem_pad[:, :D].T).astype(NP_BF16),
            "staticT": np.ascontiguousarray(st_pad.T).astype(NP_BF16),
            "msgT": np.ascontiguousarray(msg_full[:, :D].T).astype(NP_BF16),
            "lu_t": lu_pad.reshape(NP, PAIR).copy(),
            "ts_t": ts_full.reshape(NEP, PAIR).copy(),
            "cnt_t": mem_pad[:, D].reshape(NP, PAIR).copy(),
            "msgc_t": msg_full[:, D].reshape(NEP, PAIR).copy(),
            "w1a": w1a, "w1b": w1b, "w2": w2,
            "b1": b1c, "b2": b2c, "ones": ones,
        })
    return in_maps, perms, NEP


def _run(inputs, trace=False, trace_cores=None):
    in_maps, perms, NEP = _preprocess(**inputs)
    nc = _build(NEP, inputs["e_lamb"], inputs["now_time"])
    res = run_bass_kernel_spmd(nc, in_maps, core_ids=list(range(NCORES)),
                               trace=trace, trace_cores=trace_cores)
    out = np.empty((N_NODES, D), dtype=np.float32)
    for c in range(NCORES):
        out_perm = res.results[c]["outT"].T[:S].astype(np.float32)
        shard = np.empty((S, D), dtype=np.float32)
        shard[perms[c]] = out_perm
        out[c * S:(c + 1) * S] = shard
    return out, res


def kernel(**inputs) -> np.ndarray:
    out, _ = _run(inputs, trace=False)
    return out
